# revision 2
# baseline (speedup 1.0000x reference)
import numpy as np

import concourse.bacc as bacc
import concourse.bass as bass
import concourse.tile as tile
from concourse import mybir
from concourse.bass_utils import run_bass_kernel_spmd

F32 = mybir.dt.float32
BF16 = mybir.dt.float16
RELU = mybir.ActivationFunctionType.Relu

N_CORES = 8
B_FULL = 65536
D = 768
NCHUNK = 6  # 768 / 128


def build_program(per_rows: int, pw=0.5, sw=0.25, finalize=True,
                  dma_engines=("sync",), bt_per_dma=2,
                  repeat=1) -> bass.Bass:
    """One core's program: x [nb, 128, 4, 768] -> out [2, per_rows] (transposed)."""
    assert per_rows % 512 == 0
    nb = per_rows // 128
    nst = nb // 4  # super-tiles of 512 rows
    assert 4 % bt_per_dma == 0

    nc = bacc.Bacc()
    x_ext = nc.dram_tensor("x", [nb, 128, 4, D], F32, kind="ExternalInput")
    w1s_ext = nc.dram_tensor("w1s", [NCHUNK, 128, 96], BF16, kind="ExternalInput")
    w1e_ext = nc.dram_tensor("w1e", [NCHUNK, 128, 96], BF16, kind="ExternalInput")
    w2s_ext = nc.dram_tensor("w2s", [96, 48], BF16, kind="ExternalInput")
    w2e_ext = nc.dram_tensor("w2e", [96, 48], BF16, kind="ExternalInput")
    w3b_ext = nc.dram_tensor("w3b", [112, 48], BF16, kind="ExternalInput")
    w4b_ext = nc.dram_tensor("w4b", [112, 48], BF16, kind="ExternalInput")
    w5a_ext = nc.dram_tensor("w5a", [48, 64], BF16, kind="ExternalInput")
    w5b_ext = nc.dram_tensor("w5b", [48, 64], BF16, kind="ExternalInput")
    sel_ext = nc.dram_tensor("sel", [64, 2], BF16, kind="ExternalInput")
    idn_ext = nc.dram_tensor("idn", [128, 128], BF16, kind="ExternalInput")
    out_ext = nc.dram_tensor("out", [2, per_rows], F32, kind="ExternalOutput")

    # [n, 128, k, 3072] view: bt_per_dma btiles per DMA, partition-major
    xv = x_ext[:].rearrange("(n k) p s d -> n p k (s d)", k=bt_per_dma)

    with tile.TileContext(nc) as tc:
        with (
            tc.tile_pool(name="const", bufs=1) as cpool,
            tc.tile_pool(name="x", bufs=3) as xpool,
            tc.tile_pool(name="uv", bufs=3) as uvpool,
            tc.tile_pool(name="stage", bufs=2) as stpool,
            tc.tile_pool(name="chain_sb", bufs=2) as csb,
            tc.tile_pool(name="smalls", bufs=8) as smpool,
            tc.tile_pool(name="tpsum", bufs=2, space=bass.MemorySpace.PSUM) as tpsum,
            tc.tile_pool(name="cpsum", bufs=2, space=bass.MemorySpace.PSUM) as cpsum,
        ):
            # --- constants ---
            w1s_t = cpool.tile([128, NCHUNK, 96], BF16)
            w1e_t = cpool.tile([128, NCHUNK, 96], BF16)
            for c in range(NCHUNK):
                nc.gpsimd.dma_start(w1s_t[:, c, :], w1s_ext[c])
                nc.gpsimd.dma_start(w1e_t[:, c, :], w1e_ext[c])
            w2s_t = cpool.tile([96, 48], BF16)
            w2e_t = cpool.tile([96, 48], BF16)
            w3b_t = cpool.tile([112, 48], BF16)
            w4b_t = cpool.tile([112, 48], BF16)
            w5a_t = cpool.tile([48, 64], BF16)
            w5b_t = cpool.tile([48, 64], BF16)
            sel_t = cpool.tile([64, 2], BF16)
            idn_t = cpool.tile([128, 128], BF16)
            out_sb = cpool.tile([2, per_rows], F32)
            for t, e in [
                (w2s_t[:], w2s_ext), (w2e_t[:], w2e_ext),
                (w3b_t[:], w3b_ext), (w4b_t[:], w4b_ext),
                (w5a_t[:], w5a_ext), (w5b_t[:], w5b_ext),
                (sel_t[:], sel_ext), (idn_t[:], idn_ext),
            ]:
                nc.gpsimd.dma_start(t, e[:])

            stages = {}
            n_dma_per_st = 4 // bt_per_dma

            def emit_btile_group(st):
                # stage layout: [128 feat_part, 4 bt, 6 chunk, 128 row]
                stage_pair = stpool.tile([128, 4, NCHUNK, 128], BF16)
                stage_seq = stpool.tile([128, 4, NCHUNK, 128], BF16)
                stages[st] = (stage_pair, stage_seq)
                xts = []
                for g in range(n_dma_per_st):
                    eng = dma_engines[(st * n_dma_per_st + g) % len(dma_engines)]
                    xt = xpool.tile([128, bt_per_dma, 4, D], F32)
                    getattr(nc, eng).dma_start(
                        xt[:], xv[st * n_dma_per_st + g])
                    xts.append(xt)
                for bt4 in range(4):
                    xt = xts[bt4 // bt_per_dma][:, bt4 % bt_per_dma]
                    # u = x0 + x2 (pair pre-relu*2), w = u + v (seq pre-relu*4)
                    uw = uvpool.tile([128, 3, D], BF16)
                    u, v, w = uw[:, 0, :], uw[:, 1, :], uw[:, 2, :]
                    nc.vector.tensor_add(u, xt[:, 0, :], xt[:, 2, :])
                    nc.gpsimd.tensor_add(v, xt[:, 1, :], xt[:, 3, :])
                    nc.vector.tensor_add(w, u, v)
                    tp = tpsum.tile([128, 2, NCHUNK, 128], BF16)
                    for c in range(NCHUNK):
                        cs = slice(c * 128, (c + 1) * 128)
                        nc.tensor.matmul(tp[:, 0, c, :], u[:, cs], idn_t[:],
                                         is_transpose=True, start=True, stop=True)
                        nc.tensor.matmul(tp[:, 1, c, :], w[:, cs], idn_t[:],
                                         is_transpose=True, start=True, stop=True)
                    # scaled relu drains PSUM -> stage (ACT)
                    nc.scalar.activation(stage_pair[:, bt4], tp[:, 0], RELU,
                                         scale=pw)
                    nc.scalar.activation(stage_seq[:, bt4], tp[:, 1], RELU,
                                         scale=sw)

            def emit_chains(st):
                stage_pair, stage_seq = stages.pop(st)
                # L1: 4 chains (pair_s, pair_e, seq_s, seq_e)
                l1_sb = []
                for ci, (stg, w1) in enumerate([
                        (stage_pair, w1s_t), (stage_pair, w1e_t),
                        (stage_seq, w1s_t), (stage_seq, w1e_t)]):
                    l1 = cpsum.tile([96, 512], F32, tag="c")
                    for c in range(NCHUNK):
                        nc.tensor.matmul(l1[:], w1[:, c, :],
                                         stg[:, :, c, :],
                                         start=(c == 0), stop=(c == NCHUNK - 1))
                    sb = csb.tile([96, 512], BF16, tag="l1sb", bufs=6)
                    # split relu drains between ACT and DVE to balance
                    if ci % 2 == 0:
                        nc.scalar.activation(sb[:], l1[:], RELU)
                    else:
                        nc.vector.tensor_scalar_max(sb[:], l1[:], 0.0)
                    l1_sb.append(sb)
                # L2: 4 matmuls; drain into 2 merged tiles (s on parts 0:48,
                # e on parts 64:112 — engines can only start at 0/32/64/96)
                # so L3+ can use block-diagonal weights (zero rows 48:64)
                l2m = []
                for grp in range(2):  # 0: pair, 1: seq
                    merged = csb.tile([112, 512], BF16, tag="l2sb", bufs=4)
                    nc.vector.memset(merged[32:64, :], 0.0)
                    for k, w2 in enumerate((w2s_t, w2e_t)):
                        l2 = cpsum.tile([48, 512], F32, tag="c")
                        nc.tensor.matmul(l2[:], w2[:], l1_sb[2 * grp + k][:],
                                         start=True, stop=True)
                        nc.scalar.activation(merged[64 * k:64 * k + 48, :],
                                             l2[:], RELU)
                    l2m.append(merged)
                # L3: 2 block-diag matmuls ([48s|48e] -> [24s|24e]); drain
                # into one tile: parts 0:48 pair(s|e), 64:112 seq(s|e)
                l3all = csb.tile([112, 512], BF16, tag="l3sb", bufs=4)
                nc.vector.memset(l3all[32:64, :], 0.0)
                for grp in range(2):
                    l3 = cpsum.tile([48, 512], F32, tag="c")
                    nc.tensor.matmul(l3[:], w3b_t[:], l2m[grp][:],
                                     start=True, stop=True)
                    nc.scalar.activation(l3all[64 * grp:64 * grp + 48, :],
                                         l3[:], RELU)
                # L4: one block-diag matmul over all 4 chains [112 -> 48]
                l4 = cpsum.tile([48, 512], F32, tag="c")
                nc.tensor.matmul(l4[:], w4b_t[:], l3all[:],
                                 start=True, stop=True)
                l4sb = csb.tile([48, 512], BF16, tag="l4sb", bufs=4)
                nc.scalar.activation(l4sb[:], l4[:], RELU)
                # L5 as two matmuls: A holds the s-side terms (cw*sW5),
                # B the e-side sums, at matching partitions (pair at 0:2,
                # seq at 32:34) so the cross-multiply has equal bases.
                l5a = cpsum.tile([64, 512], F32, tag="c")
                nc.tensor.matmul(l5a[:], w5a_t[:], l4sb[:],
                                 start=True, stop=True)
                l5b = cpsum.tile([64, 512], F32, tag="c")
                nc.tensor.matmul(l5b[:], w5b_t[:], l4sb[:],
                                 start=True, stop=True)
                bsb = smpool.tile([64, 512], BF16, tag="bsb", bufs=2)
                nc.scalar.activation(bsb[:], l5b[:],
                                     mybir.ActivationFunctionType.Copy)
                prod = smpool.tile([64, 512], BF16, tag="prod", bufs=2)
                nc.vector.tensor_mul(prod[:], l5a[:], bsb[:])
                # final add of (pair rows 0:2) + (seq rows 32:34) via a
                # selector matmul (cross-partition adds on one SB tensor
                # are rejected by the compiler)
                fin = cpsum.tile([2, 512], F32, tag="c")
                nc.tensor.matmul(fin[:], sel_t[:], prod[:],
                                 start=True, stop=True)
                nc.scalar.activation(out_sb[:, st * 512:(st + 1) * 512],
                                     fin[:],
                                     mybir.ActivationFunctionType.Copy)

            # 1-super-tile software pipeline
            for _ in range(repeat):
                for st in range(nst + 1):
                    if st < nst:
                        emit_btile_group(st)
                    if st >= 1:
                        emit_chains(st - 1)
            nc.gpsimd.dma_start(out_ext[:], out_sb[:])

    if finalize:
        nc.finalize()
    return nc


def prep_weights(sW1, sW2, sW3, sW4, sW5, eW1, eW2, eW3, eW4, eW5,
                 s_seq, s_pair, e_seq, e_pair, cross_w):
    bf = np.float16
    s_pair = np.asarray(s_pair, np.float32)
    e_pair = np.asarray(e_pair, np.float32)
    s_seq = np.asarray(s_seq, np.float32)
    e_seq = np.asarray(e_seq, np.float32)
    cross_w = np.asarray(cross_w, np.float32)
    assert np.allclose(s_pair, e_pair) and np.allclose(s_seq, e_seq)
    assert np.allclose(s_pair, s_pair[0]) and np.allclose(s_seq, s_seq[0])
    pw = float(s_pair[0])
    sw = float(s_seq[0])
    assert pw == 0.5 and sw == 0.25, (pw, sw)
    f32 = lambda a: np.asarray(a, np.float32)
    w3b = np.zeros((112, 48), np.float32)
    w3b[0:48, 0:24] = f32(sW3).T
    w3b[64:112, 24:48] = f32(eW3).T
    w4b = np.zeros((112, 48), np.float32)
    w4b[0:24, 0:12] = f32(sW4).T
    w4b[24:48, 12:24] = f32(eW4).T
    w4b[64:88, 24:36] = f32(sW4).T
    w4b[88:112, 36:48] = f32(eW4).T
    e2 = np.repeat(f32(eW5).sum(axis=0)[:, None], 2, axis=1)  # [12, 2]
    w5a = np.zeros((48, 64), np.float32)
    w5a[0:12, 0:2] = cross_w[0] * f32(sW5).T
    w5a[24:36, 32:34] = cross_w[1] * f32(sW5).T
    w5b = np.zeros((48, 64), np.float32)
    w5b[12:24, 0:2] = e2
    w5b[36:48, 32:34] = e2
    sel = np.zeros((64, 2), np.float32)
    sel[0, 0] = sel[1, 1] = 1.0
    sel[32, 0] = sel[33, 1] = 1.0
    c = lambda a: np.ascontiguousarray(a.astype(bf))
    return {
        "w1s": c(f32(sW1).T.reshape(NCHUNK, 128, 96)),
        "w1e": c(f32(eW1).T.reshape(NCHUNK, 128, 96)),
        "w2s": c(f32(sW2).T),
        "w2e": c(f32(eW2).T),
        "w3b": c(w3b),
        "w4b": c(w4b),
        "w5a": c(w5a),
        "w5b": c(w5b),
        "sel": c(sel),
        "idn": c(np.eye(128, dtype=np.float32)),
    }


def kernel(**inputs) -> np.ndarray:
    result = np.asarray(inputs["result"], np.float32)
    B = result.shape[0]
    per = B // N_CORES
    wmap = prep_weights(**{k: np.asarray(v) for k, v in inputs.items()
                           if k != "result"})
    nc = build_program(per)
    xs = result.reshape(B // 128, 128, 4, D)
    nb = per // 128
    in_maps = []
    for k in range(N_CORES):
        m = dict(wmap)
        m["x"] = np.ascontiguousarray(xs[k * nb:(k + 1) * nb])
        in_maps.append(m)
    res = run_bass_kernel_spmd(nc, in_maps, list(range(N_CORES)))
    return np.concatenate([r["out"].T for r in res.results], axis=0)


# revision 3
# speedup vs baseline: 1.3493x; 1.3493x over previous
import numpy as np

import concourse.bacc as bacc
import concourse.bass as bass
import concourse.tile as tile
from concourse import mybir
from concourse.bass_utils import run_bass_kernel_spmd

F32 = mybir.dt.float32
BF16 = mybir.dt.float16
RELU = mybir.ActivationFunctionType.Relu

N_CORES = 8
B_FULL = 65536
D = 768
NCHUNK = 6  # 768 / 128


def build_program(per_rows: int, pw=0.5, sw=0.25, finalize=True,
                  dma_engines=("sync",), bt_per_dma=2,
                  repeat=1) -> bass.Bass:
    """One core's program: x [nb, 128, 4, 768] -> out [2, per_rows] (transposed)."""
    assert per_rows % 512 == 0
    nb = per_rows // 128
    nst = nb // 4  # super-tiles of 512 rows
    assert 4 % bt_per_dma == 0

    nc = bacc.Bacc()
    x_ext = nc.dram_tensor("x", [nb, 128, 4, D], F32, kind="ExternalInput")
    w1s_ext = nc.dram_tensor("w1s", [NCHUNK, 128, 96], BF16, kind="ExternalInput")
    w1e_ext = nc.dram_tensor("w1e", [NCHUNK, 128, 96], BF16, kind="ExternalInput")
    w2s_ext = nc.dram_tensor("w2s", [96, 48], BF16, kind="ExternalInput")
    w2e_ext = nc.dram_tensor("w2e", [96, 48], BF16, kind="ExternalInput")
    w3b_ext = nc.dram_tensor("w3b", [112, 48], BF16, kind="ExternalInput")
    w4b_ext = nc.dram_tensor("w4b", [112, 48], BF16, kind="ExternalInput")
    w5a_ext = nc.dram_tensor("w5a", [48, 64], BF16, kind="ExternalInput")
    w5b_ext = nc.dram_tensor("w5b", [48, 64], BF16, kind="ExternalInput")
    sel_ext = nc.dram_tensor("sel", [64, 2], BF16, kind="ExternalInput")
    idn_ext = nc.dram_tensor("idn", [128, 128], BF16, kind="ExternalInput")
    out_ext = nc.dram_tensor("out", [2, per_rows], F32, kind="ExternalOutput")

    # [n, 128, k, 3072] view: bt_per_dma btiles per DMA, partition-major
    xv = x_ext[:].rearrange("(n k) p s d -> n p k (s d)", k=bt_per_dma)

    with tile.TileContext(nc) as tc:
        with (
            tc.tile_pool(name="const", bufs=1) as cpool,
            tc.tile_pool(name="x", bufs=4) as xpool,
            tc.tile_pool(name="uv", bufs=3) as uvpool,
            tc.tile_pool(name="stage", bufs=2) as stpool,
            tc.tile_pool(name="chain_sb", bufs=2) as csb,
            tc.tile_pool(name="smalls", bufs=8) as smpool,
            tc.tile_pool(name="tpsum", bufs=2, space=bass.MemorySpace.PSUM) as tpsum,
            tc.tile_pool(name="cpsum", bufs=2, space=bass.MemorySpace.PSUM) as cpsum,
        ):
            # --- constants ---
            w1s_t = cpool.tile([128, NCHUNK, 96], BF16)
            w1e_t = cpool.tile([128, NCHUNK, 96], BF16)
            for c in range(NCHUNK):
                nc.gpsimd.dma_start(w1s_t[:, c, :], w1s_ext[c])
                nc.gpsimd.dma_start(w1e_t[:, c, :], w1e_ext[c])
            w2s_t = cpool.tile([96, 48], BF16)
            w2e_t = cpool.tile([96, 48], BF16)
            w3b_t = cpool.tile([112, 48], BF16)
            w4b_t = cpool.tile([112, 48], BF16)
            w5a_t = cpool.tile([48, 64], BF16)
            w5b_t = cpool.tile([48, 64], BF16)
            sel_t = cpool.tile([64, 2], BF16)
            idn_t = cpool.tile([128, 128], BF16)
            out_sb = cpool.tile([2, per_rows], F32)
            for t, e in [
                (w2s_t[:], w2s_ext), (w2e_t[:], w2e_ext),
                (w3b_t[:], w3b_ext), (w4b_t[:], w4b_ext),
                (w5a_t[:], w5a_ext), (w5b_t[:], w5b_ext),
                (sel_t[:], sel_ext), (idn_t[:], idn_ext),
            ]:
                nc.gpsimd.dma_start(t, e[:])

            stages = {}
            n_dma_per_st = 4 // bt_per_dma

            def emit_btile_group(st):
                # stage layout: [128 feat_part, 4 bt, 6 chunk, 128 row]
                stage_pair = stpool.tile([128, 4, NCHUNK, 128], BF16)
                stage_seq = stpool.tile([128, 4, NCHUNK, 128], BF16)
                stages[st] = (stage_pair, stage_seq)
                xts = []
                for g in range(n_dma_per_st):
                    eng = dma_engines[(st * n_dma_per_st + g) % len(dma_engines)]
                    xt = xpool.tile([128, bt_per_dma, 4, D], F32)
                    getattr(nc, eng).dma_start(
                        xt[:], xv[st * n_dma_per_st + g])
                    xts.append(xt)
                for bt4 in range(4):
                    xt = xts[bt4 // bt_per_dma][:, bt4 % bt_per_dma]
                    # u = x0 + x2 (pair pre-relu*2), w = u + v (seq pre-relu*4)
                    uw = uvpool.tile([128, 3, D], BF16)
                    u, v, w = uw[:, 0, :], uw[:, 1, :], uw[:, 2, :]
                    nc.vector.tensor_add(u, xt[:, 0, :], xt[:, 2, :])
                    nc.gpsimd.tensor_add(v, xt[:, 1, :], xt[:, 3, :])
                    nc.vector.tensor_add(w, u, v)
                    tp = tpsum.tile([128, 2, NCHUNK, 128], BF16)
                    for c in range(NCHUNK):
                        cs = slice(c * 128, (c + 1) * 128)
                        nc.tensor.matmul(tp[:, 0, c, :], u[:, cs], idn_t[:],
                                         is_transpose=True, start=True, stop=True)
                        nc.tensor.matmul(tp[:, 1, c, :], w[:, cs], idn_t[:],
                                         is_transpose=True, start=True, stop=True)
                    # scaled relu drains PSUM -> stage (ACT)
                    nc.scalar.activation(stage_pair[:, bt4], tp[:, 0], RELU,
                                         scale=pw)
                    nc.scalar.activation(stage_seq[:, bt4], tp[:, 1], RELU,
                                         scale=sw)

            def emit_chains(st):
                stage_pair, stage_seq = stages.pop(st)
                # L1: 4 chains (pair_s, pair_e, seq_s, seq_e)
                l1_sb = []
                for ci, (stg, w1) in enumerate([
                        (stage_pair, w1s_t), (stage_pair, w1e_t),
                        (stage_seq, w1s_t), (stage_seq, w1e_t)]):
                    l1 = cpsum.tile([96, 512], F32, tag="c")
                    for c in range(NCHUNK):
                        nc.tensor.matmul(l1[:], w1[:, c, :],
                                         stg[:, :, c, :],
                                         start=(c == 0), stop=(c == NCHUNK - 1))
                    sb = csb.tile([96, 512], BF16, tag="l1sb", bufs=6)
                    # split relu drains between ACT and DVE to balance
                    if ci % 2 == 0:
                        nc.scalar.activation(sb[:], l1[:], RELU)
                    else:
                        nc.vector.tensor_scalar_max(sb[:], l1[:], 0.0)
                    l1_sb.append(sb)
                # L2: 4 matmuls; drain into 2 merged tiles (s on parts 0:48,
                # e on parts 64:112 — engines can only start at 0/32/64/96)
                # so L3+ can use block-diagonal weights (zero rows 48:64)
                l2m = []
                for grp in range(2):  # 0: pair, 1: seq
                    merged = csb.tile([112, 512], BF16, tag="l2sb", bufs=4)
                    nc.vector.memset(merged[32:64, :], 0.0)
                    for k, w2 in enumerate((w2s_t, w2e_t)):
                        l2 = cpsum.tile([48, 512], F32, tag="c")
                        nc.tensor.matmul(l2[:], w2[:], l1_sb[2 * grp + k][:],
                                         start=True, stop=True)
                        nc.scalar.activation(merged[64 * k:64 * k + 48, :],
                                             l2[:], RELU)
                    l2m.append(merged)
                # L3: 2 block-diag matmuls ([48s|48e] -> [24s|24e]); drain
                # into one tile: parts 0:48 pair(s|e), 64:112 seq(s|e)
                l3all = csb.tile([112, 512], BF16, tag="l3sb", bufs=4)
                nc.vector.memset(l3all[32:64, :], 0.0)
                for grp in range(2):
                    l3 = cpsum.tile([48, 512], F32, tag="c")
                    nc.tensor.matmul(l3[:], w3b_t[:], l2m[grp][:],
                                     start=True, stop=True)
                    nc.scalar.activation(l3all[64 * grp:64 * grp + 48, :],
                                         l3[:], RELU)
                # L4: one block-diag matmul over all 4 chains [112 -> 48]
                l4 = cpsum.tile([48, 512], F32, tag="c")
                nc.tensor.matmul(l4[:], w4b_t[:], l3all[:],
                                 start=True, stop=True)
                l4sb = csb.tile([48, 512], BF16, tag="l4sb", bufs=4)
                nc.scalar.activation(l4sb[:], l4[:], RELU)
                # L5 as two matmuls: A holds the s-side terms (cw*sW5),
                # B the e-side sums, at matching partitions (pair at 0:2,
                # seq at 32:34) so the cross-multiply has equal bases.
                l5a = cpsum.tile([64, 512], F32, tag="c")
                nc.tensor.matmul(l5a[:], w5a_t[:], l4sb[:],
                                 start=True, stop=True)
                l5b = cpsum.tile([64, 512], F32, tag="c")
                nc.tensor.matmul(l5b[:], w5b_t[:], l4sb[:],
                                 start=True, stop=True)
                bsb = smpool.tile([64, 512], BF16, tag="bsb", bufs=2)
                nc.scalar.activation(bsb[:], l5b[:],
                                     mybir.ActivationFunctionType.Copy)
                prod = smpool.tile([64, 512], BF16, tag="prod", bufs=2)
                nc.vector.tensor_mul(prod[:], l5a[:], bsb[:])
                # final add of (pair rows 0:2) + (seq rows 32:34) via a
                # selector matmul (cross-partition adds on one SB tensor
                # are rejected by the compiler)
                fin = cpsum.tile([2, 512], F32, tag="c")
                nc.tensor.matmul(fin[:], sel_t[:], prod[:],
                                 start=True, stop=True)
                nc.scalar.activation(out_sb[:, st * 512:(st + 1) * 512],
                                     fin[:],
                                     mybir.ActivationFunctionType.Copy)

            # 1-super-tile software pipeline
            for _ in range(repeat):
                for st in range(nst + 1):
                    if st < nst:
                        emit_btile_group(st)
                    if st >= 1:
                        emit_chains(st - 1)
            nc.gpsimd.dma_start(out_ext[:], out_sb[:])

    if finalize:
        nc.finalize()
    return nc


def prep_weights(sW1, sW2, sW3, sW4, sW5, eW1, eW2, eW3, eW4, eW5,
                 s_seq, s_pair, e_seq, e_pair, cross_w):
    bf = np.float16
    s_pair = np.asarray(s_pair, np.float32)
    e_pair = np.asarray(e_pair, np.float32)
    s_seq = np.asarray(s_seq, np.float32)
    e_seq = np.asarray(e_seq, np.float32)
    cross_w = np.asarray(cross_w, np.float32)
    assert np.allclose(s_pair, e_pair) and np.allclose(s_seq, e_seq)
    assert np.allclose(s_pair, s_pair[0]) and np.allclose(s_seq, s_seq[0])
    pw = float(s_pair[0])
    sw = float(s_seq[0])
    assert pw == 0.5 and sw == 0.25, (pw, sw)
    f32 = lambda a: np.asarray(a, np.float32)
    w3b = np.zeros((112, 48), np.float32)
    w3b[0:48, 0:24] = f32(sW3).T
    w3b[64:112, 24:48] = f32(eW3).T
    w4b = np.zeros((112, 48), np.float32)
    w4b[0:24, 0:12] = f32(sW4).T
    w4b[24:48, 12:24] = f32(eW4).T
    w4b[64:88, 24:36] = f32(sW4).T
    w4b[88:112, 36:48] = f32(eW4).T
    e2 = np.repeat(f32(eW5).sum(axis=0)[:, None], 2, axis=1)  # [12, 2]
    w5a = np.zeros((48, 64), np.float32)
    w5a[0:12, 0:2] = cross_w[0] * f32(sW5).T
    w5a[24:36, 32:34] = cross_w[1] * f32(sW5).T
    w5b = np.zeros((48, 64), np.float32)
    w5b[12:24, 0:2] = e2
    w5b[36:48, 32:34] = e2
    sel = np.zeros((64, 2), np.float32)
    sel[0, 0] = sel[1, 1] = 1.0
    sel[32, 0] = sel[33, 1] = 1.0
    c = lambda a: np.ascontiguousarray(a.astype(bf))
    return {
        "w1s": c(f32(sW1).T.reshape(NCHUNK, 128, 96)),
        "w1e": c(f32(eW1).T.reshape(NCHUNK, 128, 96)),
        "w2s": c(f32(sW2).T),
        "w2e": c(f32(eW2).T),
        "w3b": c(w3b),
        "w4b": c(w4b),
        "w5a": c(w5a),
        "w5b": c(w5b),
        "sel": c(sel),
        "idn": c(np.eye(128, dtype=np.float32)),
    }


def kernel(**inputs) -> np.ndarray:
    result = np.asarray(inputs["result"], np.float32)
    B = result.shape[0]
    per = B // N_CORES
    wmap = prep_weights(**{k: np.asarray(v) for k, v in inputs.items()
                           if k != "result"})
    nc = build_program(per)
    xs = result.reshape(B // 128, 128, 4, D)
    nb = per // 128
    in_maps = []
    for k in range(N_CORES):
        m = dict(wmap)
        m["x"] = np.ascontiguousarray(xs[k * nb:(k + 1) * nb])
        in_maps.append(m)
    res = run_bass_kernel_spmd(nc, in_maps, list(range(N_CORES)))
    return np.concatenate([r["out"].T for r in res.results], axis=0)
